# revision 2
# baseline (speedup 1.0000x reference)
"""Multi-head attention (B=2, S=2048, D=1024, H=16) on 8 TRN2 NeuronCores.

Sharding: core c handles batch c//4 and head-group c%4 (4 heads each).
v2 of the kernel. Key differences from v1:

- Projections run as fp8 DoubleRow matmuls: host splits x and w into
  (hi, lo) e4m3 pairs; emitting hi*hi + lo*hi + hi*lo (dropping lo*lo,
  ~0.06% error) costs 3 DR instructions per 2 K-tiles = 75% of the bf16
  matmul cycles at full reconstruction accuracy.
- Scores also run fp8 DoubleRow: k is kept as an exact (hi, lo) pair,
  q as a single e4m3 (duplicated across the pair dim), so one DR matmul
  computes (khi + klo)^T q8 in half the bf16 cycles; only q's fp8
  quantization (~1%) enters the error budget.
- The softmax row-sum comes from a leading ones-column in the V tiles,
  so the denominator lands on PSUM PARTITION 0: reciprocal runs
  directly on it and gpsimd.partition_broadcast fans it out across
  partitions - the v1 DRAM round-trip is gone.
- qt/kt zero padding halves are initialized by broadcast DMAs from tiny
  host-provided zero tensors instead of slow gpsimd memsets (v1 lost
  ~13us of startup to those).
- Emission interleaves next-chunk projection pieces and prev-chunk
  dense pieces between attention score/AV groups so the in-order PE
  queue always has independent work while exp (scalar engine) catches
  up.
"""

import numpy as np
import ml_dtypes

import concourse.bass as bass
import concourse.tile as tile
from concourse import bacc, mybir
from concourse.bass_utils import run_bass_kernel_spmd

BF16 = mybir.dt.bfloat16
F32 = mybir.dt.float32
FP8 = mybir.dt.float8e4
NPBF16 = ml_dtypes.bfloat16
NPFP8 = ml_dtypes.float8_e4m3

D_MODEL = 1024
NH = 16
DEPTH = 64
B = 2
S = 2048
N_CORES = 8
GROUPS = 4              # head-groups (tensor parallel dimension)
HPG = NH // GROUPS      # 4 heads per core
OG = HPG * DEPTH        # 256 projection output cols per core
QC = 512                # q chunk (matmul free dim)
NQC = S // QC           # 4
KT = 128                # k tile (psum partition dim)
NKT = S // KT           # 16
DK = D_MODEL // 128     # 8 contraction tiles of 128
SC = 512                # projection s chunk
NSC = S // SC           # 4
EGRP = 2                # k-tiles per exp group (psum group tile)
HORDER = (0, 2, 1, 3)   # even heads first: their avf partition-shift DMAs
                        # start earlier and hide under the odd heads' attn

DR = mybir.MatmulPerfMode.DoubleRow

TRACE = False
TRACE_KW = {}
LAST_RESULT = None
_CACHE = {}


def _chunk(lst, n):
    return [lst[i : i + n] for i in range(0, len(lst), n)]


TAU = 16.0   # target absmax scale of fp8 q/k/v projection outputs


def _build(ktiles, n_uniq, zero_bias, cast_mults, num_devices=N_CORES):
    """Emit the bass program. ktiles[j] = [(t, lo, tri), ...] computed
    k-tiles for q-chunk j (see _classify_mask).

    cast_mults = (cq, ck, cv): constants folded into the psum->fp8/bf16
    casts that renormalize the host-side power-of-2 input scaling (see
    kernel()) to the common TAU scale. exp scale absorbs TAU^2 back out.
    """
    cq, ck, cv = cast_mults
    exp_scale = 0.125 / (TAU * TAU)
    nc = bacc.Bacc(
        "TRN2", target_bir_lowering=False, debug=False, num_devices=num_devices
    )
    # fp8 (hi, lo) pairs, hi/lo adjacent on the dim after dk so a
    # [:, dkp:dkp+2, s, :] slice is a DoubleRow operand pair
    xq = nc.dram_tensor("xq", [NSC, 128, DK, 2, SC], FP8, kind="ExternalInput").ap()
    xk = nc.dram_tensor("xk", [NSC, 128, DK, 2, SC], FP8, kind="ExternalInput").ap()
    xv = nc.dram_tensor("xv", [NSC, 128, DK, 2, SC], FP8, kind="ExternalInput").ap()
    wq = nc.dram_tensor("wq", [128, DK, 2, OG], FP8, kind="ExternalInput").ap()
    wk = nc.dram_tensor("wk", [128, DK, 2, OG], FP8, kind="ExternalInput").ap()
    wv = nc.dram_tensor("wv", [128, DK, 2, OG], FP8, kind="ExternalInput").ap()
    wd = nc.dram_tensor("wd", [128, 2, D_MODEL], BF16, kind="ExternalInput").ap()
    qb = nc.dram_tensor("qb", [128, 2], F32, kind="ExternalInput").ap()
    kb = nc.dram_tensor("kb", [128, 2], F32, kind="ExternalInput").ap()
    mk = nc.dram_tensor("mk", [128, n_uniq, KT], BF16, kind="ExternalInput").ap()
    # constants for broadcast-DMA initialization
    z8 = nc.dram_tensor("z8", [1, 2, 2, S], FP8, kind="ExternalInput").ap()
    vinit = nc.dram_tensor("vinit", [1, NKT, HPG, 64], BF16, kind="ExternalInput").ap()
    outp = nc.dram_tensor("outp", [S, D_MODEL], F32, kind="ExternalOutput").ap()

    Exp = mybir.ActivationFunctionType.Exp

    with tile.TileContext(nc) as tc:
        with (
            tc.tile_pool(name="singles", bufs=1) as singles,
            tc.tile_pool(name="xin", bufs=5) as xin,
            tc.tile_pool(name="exps", bufs=3) as exps,
            tc.tile_pool(name="small", bufs=4) as small,
            tc.tile_pool(name="bcastp", bufs=4) as bcastp,
            tc.tile_pool(name="ost", bufs=3) as ost,
            # PSUM: scores groups (2 banks x 2 bufs) + shared pool for
            # projections / AV accumulation / dense (1 bank x 4 bufs)
            tc.tile_pool(name="psc", bufs=2, space="PSUM") as psc,
            tc.tile_pool(name="ppav", bufs=4, space="PSUM") as ppav,
        ):
            wq_sb = singles.tile([128, DK, 2, OG], FP8)
            nc.sync.dma_start(wq_sb[:], wq)
            wk_sb = singles.tile([128, DK, 2, OG], FP8)
            nc.sync.dma_start(wk_sb[:], wk)
            wv_sb = singles.tile([128, DK, 2, OG], FP8)
            nc.sync.dma_start(wv_sb[:], wv)
            mk_sb = singles.tile([128, n_uniq, KT], BF16)
            nc.sync.dma_start(mk_sb[:], mk)
            if not zero_bias:
                qb_sb = singles.tile([128, 2], F32)
                nc.sync.dma_start(qb_sb[:], qb)
                kb_sb = singles.tile([128, 2], F32)
                nc.sync.dma_start(kb_sb[:], kb)
            else:
                qb_sb = kb_sb = None
            wd_sb = singles.tile([128, 2, D_MODEL], BF16)  # loaded after sc=0 DMAs

            # per-head fp8 layouts, zero-padded to K=128 (keeps the PE's HAM
            # activity monitor warm; K=64 streams never unthrottle).
            # head h occupies d-rows [(h%2)*64, (h%2)*64+64); rest zeros.
            # slot dim: qt8 = (q8, q8) duplicate; kt8 = (khi, klo).
            qt8 = singles.tile([128, 2, HPG, S], FP8)
            kt8 = singles.tile([128, 2, HPG, S], FP8)
            # [p = k%128, ktile, head, ones col + 63 zeros + 64 v cols]
            vh1 = singles.tile([128, NKT, HPG, 128], BF16)
            avf = singles.tile([128, 2, S], F32)    # unnormalized av^T
            avb = singles.tile([128, 2, S], BF16)   # normalized av^T

            # broadcast-DMA constant init (replaces v1's gpsimd memsets):
            # zero the unused parity halves of qt8/kt8 and set vh1's
            # [1, 0 x 63] column prefix per (ktile, head)
            for t8 in (qt8, kt8):
                t8v = t8.rearrange("p l (a b) s -> p l a b s", b=2)
                nc.sync.dma_start(
                    t8v[64:128, :, :, 0, :],
                    z8.to_broadcast([64, 2, 2, S]),
                )
                nc.sync.dma_start(
                    t8v[0:64, :, :, 1, :],
                    z8.to_broadcast([64, 2, 2, S]),
                )
            nc.sync.dma_start(
                vh1[:, :, :, 0:64], vinit.to_broadcast([128, NKT, HPG, 64])
            )

            x_tiles = {}

            def emit_chunk_dmas(sc):
                for key, src in (("q", xq), ("k", xk), ("v", xv)):
                    x_sb = xin.tile([128, DK, 2, SC], FP8, tag="xin")
                    nc.sync.dma_start(x_sb[:], src[sc])
                    x_tiles[(sc, key)] = x_sb

            def dr_mms(ps_slice, w_sb, x_sb, xcol):
                """hi*hi + lo*hi + hi*lo DoubleRow accumulation into psum."""
                n = DK // 2 * 3
                i = 0
                for dkp in range(0, DK, 2):
                    for ws, xs in ((0, 0), (1, 0), (0, 1)):
                        nc.tensor.matmul(
                            ps_slice,
                            lhsT=w_sb[:, dkp : dkp + 2, ws, :],
                            rhs=x_sb[:, dkp : dkp + 2, xs, xcol],
                            start=(i == 0),
                            stop=(i == n - 1),
                            perf_mode=DR,
                        )
                        i += 1

            def emit_qk_piece(sc, which, oc):
                """One oc-half (128 output cols = 2 head-halves) of the q or
                k projection for s-chunk sc, written into qt8/kt8."""
                x_sb = x_tiles[(sc, which)]
                w_sb = wq_sb if which == "q" else wk_sb
                b_sb = qb_sb if which == "q" else kb_sb
                dst = qt8 if which == "q" else kt8
                ssl = slice(sc * SC, (sc + 1) * SC)
                ps = ppav.tile([128, SC], F32, tag="ppav")
                dr_mms(
                    ps[:], w_sb[:, :, :, oc * 128 : (oc + 1) * 128],
                    x_sb, slice(None),
                )
                c = cq if which == "q" else ck
                for half in range(2):
                    rows = slice(half * 64, half * 64 + 64)
                    ch = 2 * oc + half
                    if zero_bias:
                        src, smul = ps[rows, :], c
                    else:
                        # generic path: bias (host-scaled by TAU) added via a
                        # bf16 staging tile, then quantized
                        stage = bcastp.tile(
                            [64, SC], BF16, tag="bstage", name="bstage"
                        )
                        nc.vector.tensor_scalar(
                            out=stage[:],
                            in0=ps[rows, :],
                            scalar1=float(c),
                            scalar2=b_sb[rows, oc : oc + 1],
                            op0=mybir.AluOpType.mult,
                            op1=mybir.AluOpType.add,
                        )
                        src, smul = stage[:], 1.0
                    nc.vector.tensor_scalar(
                        out=dst[rows, 0, ch, ssl],
                        in0=src,
                        scalar1=float(smul),
                        scalar2=None,
                        op0=mybir.AluOpType.mult,
                    )
                    if which == "q":
                        # duplicate slot (DoubleRow pair rhs)
                        nc.vector.tensor_scalar(
                            out=dst[rows, 1, ch, ssl],
                            in0=src,
                            scalar1=float(smul),
                            scalar2=None,
                            op0=mybir.AluOpType.mult,
                        )
                    else:
                        # residual slot: klo = scaled k - khi (exact pair)
                        nc.vector.scalar_tensor_tensor(
                            out=dst[rows, 1, ch, ssl],
                            in0=src,
                            scalar=float(smul),
                            in1=dst[rows, 0, ch, ssl],
                            op0=mybir.AluOpType.mult,
                            op1=mybir.AluOpType.subtract,
                        )

            def emit_v_piece(sc, sth):
                """One 128-row k-tile of the v projection -> vh1."""
                x_sb = x_tiles[(sc, "v")]
                st = sc * (SC // KT) + sth
                ps = ppav.tile([128, SC], F32, tag="ppav")
                ksl = slice(sth * KT, (sth + 1) * KT)
                n = DK // 2 * 3
                i = 0
                for dkp in range(0, DK, 2):
                    for xs, ws in ((0, 0), (1, 0), (0, 1)):
                        nc.tensor.matmul(
                            ps[:, :OG],
                            lhsT=x_sb[:, dkp : dkp + 2, xs, ksl],
                            rhs=wv_sb[:, dkp : dkp + 2, ws, :],
                            start=(i == 0),
                            stop=(i == n - 1),
                            perf_mode=DR,
                        )
                        i += 1
                nc.vector.tensor_scalar(
                    out=vh1[:, st, :, 64:128],
                    in0=ps[:, :OG].rearrange("p (h d) -> p h d", d=DEPTH),
                    scalar1=float(cv),
                    scalar2=None,
                    op0=mybir.AluOpType.mult,
                )

            pending_ot = {}

            def emit_dense_piece(st, oc):
                """Half (512 out cols) of a 128-row dense output block."""
                if oc == 0:
                    pending_ot[st] = ost.tile(
                        [128, D_MODEL], F32, tag="ostage", name=f"ot{st}"
                    )
                ot = pending_ot[st]
                ps = ppav.tile([128, SC], F32, tag="ppav")
                for co in range(2):
                    nc.tensor.matmul(
                        ps[:],
                        lhsT=avb[:, co, st * 128 : (st + 1) * 128],
                        rhs=wd_sb[:, co, oc * 512 : (oc + 1) * 512],
                        start=(co == 0),
                        stop=(co == 1),
                    )
                if oc == 0:
                    nc.vector.tensor_copy(
                        out=ot[:, oc * 512 : (oc + 1) * 512], in_=ps[:]
                    )
                else:
                    nc.scalar.copy(out=ot[:, oc * 512 : (oc + 1) * 512], in_=ps[:])
                    nc.sync.dma_start(outp[st * 128 : (st + 1) * 128, :], ot[:])
                    del pending_ot[st]

            def emit_attn(h, j, fillers):
                """Causal attention for head h, q-chunk j, popping one filler
                after each score/exp/AV group to keep the PE queue fed."""
                odd = h % 2
                pb = odd * 64
                ch = h // 2
                jsl = slice(j * QC, (j + 1) * QC)
                tiles = ktiles[j]
                first, last = tiles[0][0], tiles[-1][0]
                ps_av = ppav.tile([128, QC], F32, tag="ppav")
                if len(tiles) > 1:
                    # 1-tile first group primes the scores->exp->av pipeline
                    groups = [tiles[:1]] + _chunk(tiles[1:], EGRP)
                else:
                    groups = [tiles]
                for grp in groups:
                    ps_g = psc.tile([128, EGRP, QC], F32, tag="psc")
                    for r, (t, lo, tri) in enumerate(grp):
                        # cols [0, lo*128) are fully masked: never computed,
                        # never read by the av matmul below
                        nc.tensor.matmul(
                            ps_g[:, r, lo * 128 :],
                            lhsT=kt8[:, :, h, t * KT : (t + 1) * KT],
                            rhs=qt8[:, :, h, j * QC + lo * 128 : (j + 1) * QC],
                            start=True,
                            stop=True,
                            perf_mode=DR,
                        )
                    ex = exps.tile([128, EGRP, QC], BF16, tag="exps")
                    if all(lo == 0 for (t, lo, tri) in grp):
                        nc.scalar.activation(
                            out=ex[:, : len(grp), :],
                            in_=ps_g[:, : len(grp), :],
                            func=Exp,
                            scale=0.125,
                        )
                    else:
                        # diagonal tiles: exp only the computed column range
                        for r, (t, lo, tri) in enumerate(grp):
                            nc.scalar.activation(
                                out=ex[:, r, lo * 128 :],
                                in_=ps_g[:, r, lo * 128 :],
                                func=Exp,
                                scale=0.125,
                            )
                    for r, (t, lo, tri) in enumerate(grp):
                        for i, uid in tri:
                            nc.vector.tensor_mul(
                                ex[:, r, i * 128 : (i + 1) * 128],
                                ex[:, r, i * 128 : (i + 1) * 128],
                                mk_sb[:, uid, :],
                            )
                    for r, (t, lo, tri) in enumerate(grp):
                        nc.tensor.matmul(
                            ps_av[:, lo * 128 :],
                            lhsT=vh1[:, t, h, :],
                            rhs=ex[:, r, lo * 128 :],
                            start=(t == first),
                            stop=(t == last),
                        )
                    if fillers:
                        fillers.pop(0)()
                # softmax denominator: ones-column -> psum PARTITION 0 ->
                # reciprocal in place -> gpsimd partition broadcast. No DMA.
                den0 = small.tile([1, QC], F32, tag="den0")
                nc.vector.tensor_copy(out=den0[:], in_=ps_av[0:1, :])
                rec0 = small.tile([1, QC], F32, tag="rec0")
                nc.vector.reciprocal_approx_fast(rec0[:], den0[:])
                bc = bcastp.tile([128, QC], F32, tag="bc")
                nc.gpsimd.partition_broadcast(bc[:], rec0[:])
                # av rows live on psum partitions 64:128; even heads need
                # rows 0:64 of avf -> bounce via SBUF + DMA partition shift
                if odd:
                    nc.vector.tensor_copy(
                        out=avf[64:128, ch, jsl], in_=ps_av[64:128, :]
                    )
                else:
                    tmp = bcastp.tile([64, QC], F32, tag="avtmp")
                    nc.vector.tensor_copy(out=tmp[:], in_=ps_av[64:128, :])
                    nc.sync.dma_start(avf[0:64, ch, jsl], tmp[:])
                nc.vector.tensor_mul(
                    avb[pb : pb + 64, ch, jsl],
                    avf[pb : pb + 64, ch, jsl],
                    bc[pb : pb + 64, :],
                )

            # ---- interleaved emission ----
            emit_chunk_dmas(0)
            for oc in range(2):
                emit_qk_piece(0, "q", oc)
                emit_qk_piece(0, "k", oc)
            for sth in range(SC // KT):
                emit_v_piece(0, sth)
            nc.sync.dma_start(wd_sb[:], wd)  # dense-weight prefetch

            for sc in range(NSC):
                fillers = []
                if sc + 1 < NSC:
                    emit_chunk_dmas(sc + 1)
                    for oc in range(2):
                        fillers.append(
                            lambda sc=sc, oc=oc: emit_qk_piece(sc + 1, "q", oc)
                        )
                        fillers.append(
                            lambda sc=sc, oc=oc: emit_qk_piece(sc + 1, "k", oc)
                        )
                    for sth in range(SC // KT):
                        fillers.append(
                            lambda sc=sc, sth=sth: emit_v_piece(sc + 1, sth)
                        )
                if sc >= 1:
                    for st in range((sc - 1) * 4, sc * 4):
                        for oc in range(2):
                            fillers.append(
                                lambda st=st, oc=oc: emit_dense_piece(st, oc)
                            )
                for h in HORDER:
                    emit_attn(h, sc, fillers)
                for f in fillers:
                    f()
            for st in range((NSC - 1) * 4, NKT):
                for oc in range(2):
                    emit_dense_piece(st, oc)

    nc.compile()
    return nc


def _classify_mask(mask):
    """Classify 128(k) x 128(q) score blocks from the actual mask contents.

    Returns (ktiles, mk_arr):
      ktiles[j]: list of (t, lo, tri) per computed k-tile for q-chunk j:
        lo: first kept 128-col block within the 512-wide q-chunk (cols
            [0, lo*128) are fully masked and simply never computed/read)
        tri: [(col_block, uid), ...] 128-col blocks needing a factor mult
      mk_arr: [128, NU, 128] bf16 multiplicative factors exp(-1e9*m/8)
    """
    m2 = np.asarray(mask, dtype=np.float32).reshape(S, S)
    F = np.exp(m2 * np.float32(-1.25e8))  # exp(-1e9*m/8); 0/1 masks -> 0/1
    if (F.max(axis=1) == 0.0).any():
        raise RuntimeError("mask has fully-masked rows; unsupported")
    blocks = F.reshape(NKT, 128, NKT, 128)  # [qi, qr, t, kr]
    kept = (blocks == 1.0).all(axis=(1, 3))  # [qi, t]
    skip = (blocks == 0.0).all(axis=(1, 3))

    NB = QC // 128  # 128-col blocks per q-chunk
    ktiles = []
    uniq = {}
    mk_tiles = []

    def factor_uid(qi, t):
        fb = np.ascontiguousarray(
            F[qi * 128 : (qi + 1) * 128, t * KT : (t + 1) * KT].T
        ).astype(NPBF16)
        key = fb.tobytes()
        if key not in uniq:
            uniq[key] = len(mk_tiles)
            mk_tiles.append(fb)
        return uniq[key]

    for j in range(NQC):
        qis = list(range(j * NB, (j + 1) * NB))
        tl = []
        for t in range(NKT):
            stats = [
                "k" if kept[qi, t] else ("s" if skip[qi, t] else "m")
                for qi in qis
            ]
            if all(s == "s" for s in stats):
                continue
            lo = next(i for i, s in enumerate(stats) if s != "s")
            tri = []
            for i in range(lo, NB):
                if stats[i] == "k":
                    continue
                # mixed OR interior skip (multiply by its factor / zeros)
                tri.append((i, factor_uid(qis[i], t)))
            tl.append((t, lo, tri))
        if not tl:
            raise RuntimeError("q-chunk with no kept k-tiles; unsupported")
        # the first computed tile must span the full chunk (av 'start' MM)
        if tl[0][1] != 0:
            t0, _, tri0 = tl[0]
            tri0 = [(i, u) for i, u in tri0]
            have = {i for i, _ in tri0}
            for i in range(tl[0][1]):
                if i not in have:
                    tri0.append((i, factor_uid(qis[i], t0)))
            tl[0] = (t0, 0, sorted(tri0))
        ktiles.append(tl)
    if not mk_tiles:
        mk_tiles.append(np.ones((128, KT), dtype=NPBF16))
    mk_arr = np.ascontiguousarray(np.stack(mk_tiles, axis=0).transpose(1, 0, 2))
    return ktiles, mk_arr


def _pow2_scale(a):
    """Power-of-2 scale s such that absmax(a * s) ~= TAU. Keeps both the
    e4m3 hi value and its residual lo well inside the normal range."""
    am = float(np.abs(a).max())
    if am == 0.0:
        return 1.0
    return float(2.0 ** np.floor(np.log2(TAU / am)))


def _hi_lo(a):
    """f32 array -> (hi, lo) e4m3 pair with hi + lo ~= a (pre-scaled)."""
    hi = a.astype(NPFP8)
    lo = (a - hi.astype(np.float32)).astype(NPFP8)
    return hi, lo


def _xt_prep(x, s):
    """[S, D] f32 -> [NSC, 128, DK, 2, SC] fp8 (hi, lo), d-major."""
    xt = np.ascontiguousarray(x.T) * np.float32(s)  # [D, S] f32
    hi, lo = _hi_lo(xt)
    a = np.stack([hi, lo], axis=1)  # [D, 2, S]
    a = a.reshape(DK, 128, 2, NSC, SC).transpose(3, 1, 0, 2, 4)
    return np.ascontiguousarray(a)


def kernel(v, k, q, mask, wq_w, wq_b, wk_w, wk_b, wv_w, wv_b, dense_w, dense_b):
    global LAST_RESULT
    v = np.asarray(v, dtype=np.float32)
    k = np.asarray(k, dtype=np.float32)
    q = np.asarray(q, dtype=np.float32)
    mask = np.asarray(mask, dtype=np.float32)
    wq_w = np.asarray(wq_w, dtype=np.float32)
    wk_w = np.asarray(wk_w, dtype=np.float32)
    wv_w = np.asarray(wv_w, dtype=np.float32)
    dense_w = np.asarray(dense_w, dtype=np.float32)
    wq_b = np.asarray(wq_b, dtype=np.float32)
    wk_b = np.asarray(wk_b, dtype=np.float32)
    wv_b = np.asarray(wv_b, dtype=np.float32)
    dense_b = np.asarray(dense_b, dtype=np.float32)

    ktiles, mk_arr = _classify_mask(mask)
    zero_bias = not (np.any(wq_b) or np.any(wk_b))

    # host-side power-of-2 scaling so every hi/lo e4m3 pair stays in the
    # normal range; the device casts renormalize psum (scale sx*sw) back
    # to the common TAU scale
    sxq, sxk, sxv = _pow2_scale(q), _pow2_scale(k), _pow2_scale(v)
    swq, swk, swv = _pow2_scale(wq_w), _pow2_scale(wk_w), _pow2_scale(wv_w)
    cast_mults = (
        TAU / (sxq * swq),
        TAU / (sxk * swk),
        TAU / (sxv * swv),
    )

    key = (
        tuple(tuple((t, lo, tuple(tri)) for t, lo, tri in tl) for tl in ktiles),
        mk_arr.shape[1],
        zero_bias,
        cast_mults,
    )
    if key not in _CACHE:
        _CACHE[key] = _build(ktiles, mk_arr.shape[1], zero_bias, cast_mults)
    nc = _CACHE[key]

    # per-batch inputs (shared by the 4 cores of each batch)
    xq_b = [_xt_prep(q[b], sxq) for b in range(B)]
    xk_b = [_xt_prep(k[b], sxk) for b in range(B)]
    xv_b = [_xt_prep(v[b], sxv) for b in range(B)]

    # per-group weights: [D, OG] hi/lo pairs -> [128, DK, 2, OG]
    def wslice(w, g, s):
        ws = np.ascontiguousarray(w[g * OG : (g + 1) * OG, :].T) * np.float32(s)
        hi, lo = _hi_lo(ws)
        a = np.stack([hi, lo], axis=1)  # [D, 2, OG]
        return np.ascontiguousarray(
            a.reshape(DK, 128, 2, OG).transpose(1, 0, 2, 3)
        )

    def bslice(b_, g):
        # biases enter after the cast renormalization -> scale by TAU
        return np.ascontiguousarray(
            b_[g * OG : (g + 1) * OG].astype(np.float32).reshape(2, 128).T
        ) * np.float32(TAU)

    wq_g = [wslice(wq_w, g, swq) for g in range(GROUPS)]
    wk_g = [wslice(wk_w, g, swk) for g in range(GROUPS)]
    wv_g = [wslice(wv_w, g, swv) for g in range(GROUPS)]
    qb_g = [bslice(wq_b, g) for g in range(GROUPS)]
    kb_g = [bslice(wk_b, g) for g in range(GROUPS)]
    wd_g = []
    for g in range(GROUPS):
        # avb carries the TAU scale of the v path; fold 1/TAU into wd
        ds = (dense_w[:, g * OG : (g + 1) * OG].T / np.float32(TAU)).astype(
            NPBF16
        )  # [OG, D]
        wd_g.append(
            np.ascontiguousarray(ds.reshape(2, 128, D_MODEL).transpose(1, 0, 2))
        )

    z8 = np.zeros((1, 2, 2, S), dtype=NPFP8)
    vinit = np.zeros((1, NKT, HPG, 64), dtype=NPBF16)
    vinit[:, :, :, 0] = 1.0

    in_maps = []
    for c in range(N_CORES):
        b, g = c // GROUPS, c % GROUPS
        in_maps.append(
            {
                "xq": xq_b[b],
                "xk": xk_b[b],
                "xv": xv_b[b],
                "wq": wq_g[g],
                "wk": wk_g[g],
                "wv": wv_g[g],
                "wd": wd_g[g],
                "qb": qb_g[g],
                "kb": kb_g[g],
                "mk": mk_arr,
                "z8": z8,
                "vinit": vinit,
            }
        )

    kw = dict(trace=True, **TRACE_KW) if TRACE else {}
    res = run_bass_kernel_spmd(nc, in_maps, core_ids=list(range(N_CORES)), **kw)
    LAST_RESULT = res

    corr = dense_w @ wv_b + dense_b  # v-bias pushed through dense, + dense bias
    out = np.empty((B, S, D_MODEL), dtype=np.float32)
    for b in range(B):
        acc = np.zeros((S, D_MODEL), dtype=np.float32)
        for g in range(GROUPS):
            acc += res.results[b * GROUPS + g]["outp"]
        out[b] = acc + corr
    return out


# revision 3
# speedup vs baseline: 1.2795x; 1.2795x over previous
"""Multi-head attention (B=2, S=2048, D=1024, H=16) on 8 TRN2 NeuronCores.

Sharding: core c handles batch c//4 and head-group c%4 (4 heads each).
Host pre-transposes inputs/weights to d-major bf16; each core computes
its 4 heads' projections, causal attention, and a partial (row-parallel)
dense output [S, D] which the host sums across the 4 cores of each batch.

v3 structural design (all matmuls bf16 - fp8 DoubleRow was measured to
give zero per-column speedup on this hardware):

- Scores are computed transposed ([k, q] layout) with K zero-padded to
  128 so the PE activity monitor stays unthrottled; no on-chip
  transposes anywhere.
- The V tiles carry a leading ones-column (then 63 zeros, then the 64 v
  columns), so the softmax row-sum lands on PSUM PARTITION 0 and av on
  partitions 64:128: the reciprocal runs directly on partition 0 and
  gpsimd.partition_broadcast fans it across partitions - no DRAM
  round-trip anywhere in the normalize chain.
- qt/kt zero-padding halves and vh1 constants are initialized by
  vector/gpsimd memsets split across both engines at startup.
- Emission interleaves next-chunk projection pieces and prev-chunk dense
  pieces between attention score/AV groups ("fillers") so the in-order
  PE queue always has independent work while exp (scalar) catches up.
- Masking is applied as a multiplicative factor on the exp'd scores;
  fully-masked 128-col blocks are never computed, partially-masked ones
  are trimmed at emit time from the actual mask contents.
"""

import numpy as np
import ml_dtypes

import concourse.bass as bass
import concourse.tile as tile
from concourse import bacc, mybir
from concourse.bass_utils import run_bass_kernel_spmd

BF16 = mybir.dt.bfloat16
F32 = mybir.dt.float32
FP8 = mybir.dt.float8e4
NPBF16 = ml_dtypes.bfloat16
NPFP8 = ml_dtypes.float8_e4m3

D_MODEL = 1024
NH = 16
DEPTH = 64
B = 2
S = 2048
N_CORES = 8
GROUPS = 4              # head-groups (tensor parallel dimension)
HPG = NH // GROUPS      # 4 heads per core
OG = HPG * DEPTH        # 256 projection output cols per core
QC = 512                # q chunk (matmul free dim)
NQC = S // QC           # 4
KT = 128                # k tile (psum partition dim)
NKT = S // KT           # 16
DK = D_MODEL // 128     # 8 contraction tiles of 128
SC = 512                # projection s chunk
NSC = S // SC           # 4
EGRP = 2                # k-tiles per exp group (psum group tile)
HORDER = (0, 2, 1, 3)   # even heads first: their avf partition-shift DMAs
                        # start earlier and hide under the odd heads' attn

TRACE = False
TRACE_KW = {}
LAST_RESULT = None
_CACHE = {}


def _chunk(lst, n):
    return [lst[i : i + n] for i in range(0, len(lst), n)]


def _build(ktiles, n_uniq, zero_bias, num_devices=N_CORES):
    """Emit the bass program. ktiles[j] = [(t, lo, tri), ...] computed
    k-tiles for q-chunk j (see _classify_mask)."""
    exp_scale = 0.125
    nc = bacc.Bacc(
        "TRN2", target_bir_lowering=False, debug=False, num_devices=num_devices
    )
    xq = nc.dram_tensor("xq", [NSC, 128, DK, SC], BF16, kind="ExternalInput").ap()
    xk = nc.dram_tensor("xk", [NSC, 128, DK, SC], BF16, kind="ExternalInput").ap()
    xv = nc.dram_tensor("xv", [NSC, 128, DK, SC], BF16, kind="ExternalInput").ap()
    wq = nc.dram_tensor("wq", [128, DK, OG], BF16, kind="ExternalInput").ap()
    wk = nc.dram_tensor("wk", [128, DK, OG], BF16, kind="ExternalInput").ap()
    wv = nc.dram_tensor("wv", [128, DK, OG], BF16, kind="ExternalInput").ap()
    wd = nc.dram_tensor("wd", [128, 2, D_MODEL], BF16, kind="ExternalInput").ap()
    qb = nc.dram_tensor("qb", [128, 2], F32, kind="ExternalInput").ap()
    kb = nc.dram_tensor("kb", [128, 2], F32, kind="ExternalInput").ap()
    mk = nc.dram_tensor("mk", [128, n_uniq, KT], BF16, kind="ExternalInput").ap()
    outp = nc.dram_tensor("outp", [S, D_MODEL], F32, kind="ExternalOutput").ap()

    Exp = mybir.ActivationFunctionType.Exp

    with tile.TileContext(nc) as tc:
        with (
            tc.tile_pool(name="singles", bufs=1) as singles,
            tc.tile_pool(name="xin", bufs=5) as xin,
            tc.tile_pool(name="exps", bufs=3) as exps,
            tc.tile_pool(name="small", bufs=4) as small,
            tc.tile_pool(name="bcastp", bufs=4) as bcastp,
            tc.tile_pool(name="ost", bufs=3) as ost,
            # PSUM: scores groups (2 banks x 2 bufs) + shared pool for
            # projections / AV accumulation / dense (1 bank x 4 bufs)
            tc.tile_pool(name="psc", bufs=2, space="PSUM") as psc,
            tc.tile_pool(name="ppav", bufs=4, space="PSUM") as ppav,
        ):
            wq_sb = singles.tile([128, DK, OG], BF16)
            nc.sync.dma_start(wq_sb[:], wq)
            wk_sb = singles.tile([128, DK, OG], BF16)
            nc.sync.dma_start(wk_sb[:], wk)
            wv_sb = singles.tile([128, DK, OG], BF16)
            nc.sync.dma_start(wv_sb[:], wv)
            mk_sb = singles.tile([128, n_uniq, KT], BF16)
            nc.sync.dma_start(mk_sb[:], mk)
            if not zero_bias:
                qb_sb = singles.tile([128, 2], F32)
                nc.sync.dma_start(qb_sb[:], qb)
                kb_sb = singles.tile([128, 2], F32)
                nc.sync.dma_start(kb_sb[:], kb)
            else:
                qb_sb = kb_sb = None
            wd_sb = singles.tile([128, 2, D_MODEL], BF16)  # loaded after sc=0 DMAs

            # per-head layouts, zero-padded to K=128 (keeps the PE's HAM
            # activity monitor warm; K=64 streams never unthrottle).
            # head h occupies d-rows [(h%2)*64, (h%2)*64+64); rest zeros.
            qt = singles.tile([128, HPG, S], BF16)
            kt_ = singles.tile([128, HPG, S], BF16)
            # [p = k%128, ktile, head, ones col + 63 zeros + 64 v cols]
            vh1 = singles.tile([128, NKT, HPG, 128], BF16)
            avf = singles.tile([128, 2, S], F32)    # unnormalized av^T
            avb = singles.tile([128, 2, S], BF16)   # normalized av^T

            # constant init, split across the two idle-at-startup engines;
            # even-head pads first (head 0's scores need them earliest)
            qtv = qt.rearrange("p (a b) s -> p a b s", b=2)
            ktv = kt_.rearrange("p (a b) s -> p a b s", b=2)
            nc.vector.memset(qtv[64:128, :, 0, :], 0.0)
            nc.vector.memset(ktv[64:128, :, 0, :], 0.0)
            nc.gpsimd.memset(qtv[0:64, :, 1, :], 0.0)
            nc.gpsimd.memset(ktv[0:64, :, 1, :], 0.0)
            nc.vector.memset(vh1[:, :, :, 1:64], 0.0)
            nc.gpsimd.memset(vh1[:, :, :, 0:1], 1.0)

            x_tiles = {}

            def emit_chunk_dmas(sc):
                for key, src in (("q", xq), ("k", xk), ("v", xv)):
                    x_sb = xin.tile([128, DK, SC], BF16, tag="xin")
                    nc.sync.dma_start(x_sb[:], src[sc])
                    x_tiles[(sc, key)] = x_sb

            def emit_qk_piece(sc, which, oc):
                """One oc-half (128 output cols = 2 head-halves) of the q or
                k projection for s-chunk sc, written into qt/kt_."""
                x_sb = x_tiles[(sc, which)]
                w_sb = wq_sb if which == "q" else wk_sb
                b_sb = qb_sb if which == "q" else kb_sb
                dst = qt if which == "q" else kt_
                ssl = slice(sc * SC, (sc + 1) * SC)
                ps = ppav.tile([128, SC], F32, tag="ppav")
                for dk in range(DK):
                    nc.tensor.matmul(
                        ps[:],
                        lhsT=w_sb[:, dk, oc * 128 : (oc + 1) * 128],
                        rhs=x_sb[:, dk, :],
                        start=(dk == 0),
                        stop=(dk == DK - 1),
                    )
                for half in range(2):
                    rows = slice(half * 64, half * 64 + 64)
                    ch = 2 * oc + half
                    if zero_bias:
                        nc.vector.tensor_copy(
                            out=dst[rows, ch, ssl], in_=ps[rows, :]
                        )
                    else:
                        nc.vector.tensor_scalar(
                            out=dst[rows, ch, ssl],
                            in0=ps[rows, :],
                            scalar1=b_sb[rows, oc : oc + 1],
                            scalar2=None,
                            op0=mybir.AluOpType.add,
                        )

            def emit_v_piece(sc, sth):
                """One 128-row k-tile of the v projection -> vh1."""
                x_sb = x_tiles[(sc, "v")]
                st = sc * (SC // KT) + sth
                ps = ppav.tile([128, SC], F32, tag="ppav")
                ksl = slice(sth * KT, (sth + 1) * KT)
                for dk in range(DK):
                    nc.tensor.matmul(
                        ps[:, :OG],
                        lhsT=x_sb[:, dk, ksl],
                        rhs=wv_sb[:, dk, :],
                        start=(dk == 0),
                        stop=(dk == DK - 1),
                    )
                nc.vector.tensor_copy(
                    out=vh1[:, st, :, 64:128],
                    in_=ps[:, :OG].rearrange("p (h d) -> p h d", d=DEPTH),
                )

            pending_ot = {}

            def emit_dense_piece(st, oc):
                """Half (512 out cols) of a 128-row dense output block."""
                if oc == 0:
                    pending_ot[st] = ost.tile(
                        [128, D_MODEL], F32, tag="ostage", name=f"ot{st}"
                    )
                ot = pending_ot[st]
                ps = ppav.tile([128, SC], F32, tag="ppav")
                for co in range(2):
                    nc.tensor.matmul(
                        ps[:],
                        lhsT=avb[:, co, st * 128 : (st + 1) * 128],
                        rhs=wd_sb[:, co, oc * 512 : (oc + 1) * 512],
                        start=(co == 0),
                        stop=(co == 1),
                    )
                if oc == 0:
                    nc.vector.tensor_copy(
                        out=ot[:, oc * 512 : (oc + 1) * 512], in_=ps[:]
                    )
                else:
                    nc.scalar.copy(out=ot[:, oc * 512 : (oc + 1) * 512], in_=ps[:])
                    nc.sync.dma_start(outp[st * 128 : (st + 1) * 128, :], ot[:])
                    del pending_ot[st]

            def emit_attn(h, j, fillers):
                """Causal attention for head h, q-chunk j, popping one filler
                after each score/exp/AV group to keep the PE queue fed."""
                odd = h % 2
                pb = odd * 64
                ch = h // 2
                jsl = slice(j * QC, (j + 1) * QC)
                tiles = ktiles[j]
                first, last = tiles[0][0], tiles[-1][0]
                ps_av = ppav.tile([128, QC], F32, tag="ppav")
                if len(tiles) > 1:
                    # 1-tile first group primes the scores->exp->av pipeline
                    groups = [tiles[:1]] + _chunk(tiles[1:], EGRP)
                else:
                    groups = [tiles]
                for grp in groups:
                    ps_g = psc.tile([128, EGRP, QC], F32, tag="psc")
                    for r, (t, lo, tri) in enumerate(grp):
                        # cols [0, lo*128) are fully masked: never computed,
                        # never read by the av matmul below
                        nc.tensor.matmul(
                            ps_g[:, r, lo * 128 :],
                            lhsT=kt_[:, h, t * KT : (t + 1) * KT],
                            rhs=qt[:, h, j * QC + lo * 128 : (j + 1) * QC],
                            start=True,
                            stop=True,
                        )
                    ex = exps.tile([128, EGRP, QC], BF16, tag="exps")
                    if all(lo == 0 for (t, lo, tri) in grp):
                        nc.scalar.activation(
                            out=ex[:, : len(grp), :],
                            in_=ps_g[:, : len(grp), :],
                            func=Exp,
                            scale=0.125,
                        )
                    else:
                        # diagonal tiles: exp only the computed column range
                        for r, (t, lo, tri) in enumerate(grp):
                            nc.scalar.activation(
                                out=ex[:, r, lo * 128 :],
                                in_=ps_g[:, r, lo * 128 :],
                                func=Exp,
                                scale=0.125,
                            )
                    for r, (t, lo, tri) in enumerate(grp):
                        for i, uid in tri:
                            nc.vector.tensor_mul(
                                ex[:, r, i * 128 : (i + 1) * 128],
                                ex[:, r, i * 128 : (i + 1) * 128],
                                mk_sb[:, uid, :],
                            )
                    for r, (t, lo, tri) in enumerate(grp):
                        nc.tensor.matmul(
                            ps_av[:, lo * 128 :],
                            lhsT=vh1[:, t, h, :],
                            rhs=ex[:, r, lo * 128 :],
                            start=(t == first),
                            stop=(t == last),
                        )
                    if fillers:
                        fillers.pop(0)()
                # softmax denominator: ones-column -> psum PARTITION 0 ->
                # reciprocal in place -> gpsimd partition broadcast. No DMA.
                den0 = small.tile([1, QC], F32, tag="den0")
                nc.vector.tensor_copy(out=den0[:], in_=ps_av[0:1, :])
                rec0 = small.tile([1, QC], F32, tag="rec0")
                nc.vector.reciprocal_approx_fast(rec0[:], den0[:])
                bc = bcastp.tile([128, QC], F32, tag="bc")
                nc.gpsimd.partition_broadcast(bc[:], rec0[:])
                # av rows live on psum partitions 64:128; even heads need
                # rows 0:64 of avf -> bounce via SBUF + DMA partition shift
                if odd:
                    nc.vector.tensor_copy(
                        out=avf[64:128, ch, jsl], in_=ps_av[64:128, :]
                    )
                else:
                    tmp = bcastp.tile([64, QC], F32, tag="avtmp")
                    nc.vector.tensor_copy(out=tmp[:], in_=ps_av[64:128, :])
                    nc.sync.dma_start(avf[0:64, ch, jsl], tmp[:])
                nc.vector.tensor_mul(
                    avb[pb : pb + 64, ch, jsl],
                    avf[pb : pb + 64, ch, jsl],
                    bc[pb : pb + 64, :],
                )

            # ---- interleaved emission ----
            emit_chunk_dmas(0)
            for oc in range(2):
                emit_qk_piece(0, "q", oc)
                emit_qk_piece(0, "k", oc)
            for sth in range(SC // KT):
                emit_v_piece(0, sth)
            nc.sync.dma_start(wd_sb[:], wd)  # dense-weight prefetch

            for sc in range(NSC):
                fillers = []
                if sc + 1 < NSC:
                    emit_chunk_dmas(sc + 1)
                    for oc in range(2):
                        fillers.append(
                            lambda sc=sc, oc=oc: emit_qk_piece(sc + 1, "q", oc)
                        )
                        fillers.append(
                            lambda sc=sc, oc=oc: emit_qk_piece(sc + 1, "k", oc)
                        )
                    for sth in range(SC // KT):
                        fillers.append(
                            lambda sc=sc, sth=sth: emit_v_piece(sc + 1, sth)
                        )
                if sc >= 1:
                    for st in range((sc - 1) * 4, sc * 4):
                        for oc in range(2):
                            fillers.append(
                                lambda st=st, oc=oc: emit_dense_piece(st, oc)
                            )
                for h in HORDER:
                    emit_attn(h, sc, fillers)
                for f in fillers:
                    f()
            for st in range((NSC - 1) * 4, NKT):
                for oc in range(2):
                    emit_dense_piece(st, oc)

    nc.compile()
    return nc


def _classify_mask(mask):
    """Classify 128(k) x 128(q) score blocks from the actual mask contents.

    Returns (ktiles, mk_arr):
      ktiles[j]: list of (t, lo, tri) per computed k-tile for q-chunk j:
        lo: first kept 128-col block within the 512-wide q-chunk (cols
            [0, lo*128) are fully masked and simply never computed/read)
        tri: [(col_block, uid), ...] 128-col blocks needing a factor mult
      mk_arr: [128, NU, 128] bf16 multiplicative factors exp(-1e9*m/8)
    """
    m2 = np.asarray(mask, dtype=np.float32).reshape(S, S)
    F = np.exp(m2 * np.float32(-1.25e8))  # exp(-1e9*m/8); 0/1 masks -> 0/1
    if (F.max(axis=1) == 0.0).any():
        raise RuntimeError("mask has fully-masked rows; unsupported")
    blocks = F.reshape(NKT, 128, NKT, 128)  # [qi, qr, t, kr]
    kept = (blocks == 1.0).all(axis=(1, 3))  # [qi, t]
    skip = (blocks == 0.0).all(axis=(1, 3))

    NB = QC // 128  # 128-col blocks per q-chunk
    ktiles = []
    uniq = {}
    mk_tiles = []

    def factor_uid(qi, t):
        fb = np.ascontiguousarray(
            F[qi * 128 : (qi + 1) * 128, t * KT : (t + 1) * KT].T
        ).astype(NPBF16)
        key = fb.tobytes()
        if key not in uniq:
            uniq[key] = len(mk_tiles)
            mk_tiles.append(fb)
        return uniq[key]

    for j in range(NQC):
        qis = list(range(j * NB, (j + 1) * NB))
        tl = []
        for t in range(NKT):
            stats = [
                "k" if kept[qi, t] else ("s" if skip[qi, t] else "m")
                for qi in qis
            ]
            if all(s == "s" for s in stats):
                continue
            lo = next(i for i, s in enumerate(stats) if s != "s")
            tri = []
            for i in range(lo, NB):
                if stats[i] == "k":
                    continue
                # mixed OR interior skip (multiply by its factor / zeros)
                tri.append((i, factor_uid(qis[i], t)))
            tl.append((t, lo, tri))
        if not tl:
            raise RuntimeError("q-chunk with no kept k-tiles; unsupported")
        # the first computed tile must span the full chunk (av 'start' MM)
        if tl[0][1] != 0:
            t0, _, tri0 = tl[0]
            tri0 = [(i, u) for i, u in tri0]
            have = {i for i, _ in tri0}
            for i in range(tl[0][1]):
                if i not in have:
                    tri0.append((i, factor_uid(qis[i], t0)))
            tl[0] = (t0, 0, sorted(tri0))
        ktiles.append(tl)
    if not mk_tiles:
        mk_tiles.append(np.ones((128, KT), dtype=NPBF16))
    mk_arr = np.ascontiguousarray(np.stack(mk_tiles, axis=0).transpose(1, 0, 2))
    return ktiles, mk_arr


def _xt_prep(x):
    """[S, D] f32 -> [NSC, 128, DK, SC] bf16, d-major, contiguous S-quarters."""
    xt = x.T.astype(NPBF16)  # [D, S]
    a = xt.reshape(DK, 128, NSC, SC).transpose(2, 1, 0, 3)
    return np.ascontiguousarray(a)


def kernel(v, k, q, mask, wq_w, wq_b, wk_w, wk_b, wv_w, wv_b, dense_w, dense_b):
    global LAST_RESULT
    v = np.asarray(v, dtype=np.float32)
    k = np.asarray(k, dtype=np.float32)
    q = np.asarray(q, dtype=np.float32)
    mask = np.asarray(mask, dtype=np.float32)
    wq_w = np.asarray(wq_w, dtype=np.float32)
    wk_w = np.asarray(wk_w, dtype=np.float32)
    wv_w = np.asarray(wv_w, dtype=np.float32)
    dense_w = np.asarray(dense_w, dtype=np.float32)
    wq_b = np.asarray(wq_b, dtype=np.float32)
    wk_b = np.asarray(wk_b, dtype=np.float32)
    wv_b = np.asarray(wv_b, dtype=np.float32)
    dense_b = np.asarray(dense_b, dtype=np.float32)

    ktiles, mk_arr = _classify_mask(mask)
    zero_bias = not (np.any(wq_b) or np.any(wk_b))
    key = (
        tuple(tuple((t, lo, tuple(tri)) for t, lo, tri in tl) for tl in ktiles),
        mk_arr.shape[1],
        zero_bias,
    )
    if key not in _CACHE:
        _CACHE[key] = _build(ktiles, mk_arr.shape[1], zero_bias)
    nc = _CACHE[key]

    # per-batch inputs (shared by the 4 cores of each batch)
    xq_b = [_xt_prep(q[b]) for b in range(B)]
    xk_b = [_xt_prep(k[b]) for b in range(B)]
    xv_b = [_xt_prep(v[b]) for b in range(B)]

    # per-group weights
    def wslice(w, g):
        ws = w[g * OG : (g + 1) * OG, :].T.astype(NPBF16)  # [D, OG]
        return np.ascontiguousarray(ws.reshape(DK, 128, OG).transpose(1, 0, 2))

    def bslice(b_, g):
        return np.ascontiguousarray(
            b_[g * OG : (g + 1) * OG].astype(np.float32).reshape(2, 128).T
        )

    wq_g = [wslice(wq_w, g) for g in range(GROUPS)]
    wk_g = [wslice(wk_w, g) for g in range(GROUPS)]
    wv_g = [wslice(wv_w, g) for g in range(GROUPS)]
    qb_g = [bslice(wq_b, g) for g in range(GROUPS)]
    kb_g = [bslice(wk_b, g) for g in range(GROUPS)]
    wd_g = []
    for g in range(GROUPS):
        ds = dense_w[:, g * OG : (g + 1) * OG].T.astype(NPBF16)  # [OG, D]
        wd_g.append(
            np.ascontiguousarray(ds.reshape(2, 128, D_MODEL).transpose(1, 0, 2))
        )

    in_maps = []
    for c in range(N_CORES):
        b, g = c // GROUPS, c % GROUPS
        in_maps.append(
            {
                "xq": xq_b[b],
                "xk": xk_b[b],
                "xv": xv_b[b],
                "wq": wq_g[g],
                "wk": wk_g[g],
                "wv": wv_g[g],
                "wd": wd_g[g],
                "qb": qb_g[g],
                "kb": kb_g[g],
                "mk": mk_arr,
            }
        )

    kw = dict(trace=True, **TRACE_KW) if TRACE else {}
    res = run_bass_kernel_spmd(nc, in_maps, core_ids=list(range(N_CORES)), **kw)
    LAST_RESULT = res

    corr = dense_w @ wv_b + dense_b  # v-bias pushed through dense, + dense bias
    out = np.empty((B, S, D_MODEL), dtype=np.float32)
    for b in range(B):
        acc = np.zeros((S, D_MODEL), dtype=np.float32)
        for g in range(GROUPS):
            acc += res.results[b * GROUPS + g]["outp"]
        out[b] = acc + corr
    return out
